# revision 1
# baseline (speedup 1.0000x reference)
"""Trainium2 Bass kernel for batched FK chain with tanh-MLP joint correction.

Math: per batch row,
    corr = tanh MLP_{7-15-15-7}(joints);  th = joints + off + corr
    M_j = DH(alpha_j, a_j, d_j, th_j);    out = (M_0 @ ... @ M_6)[:3, 3]
Factorization: M_j = A_j @ Rz(th_j) with A_j constant, and col 3 of M_6 is
constant, so the chain is 6 steps of (z-rotation + constant affine) on a
3-vector.

Distribution: pure data parallel, batch/8 = 32768 rows per NeuronCore.

Per-core pipeline (two batch halves, pipelined against each other):
  - host packs a feature-major image [128, 2048]: partition q = 64h+8k+g
    (16 batch groups x 7 features, 8 gap rows), free n = 128c+p,
    batch row b = 256p + 16c + 8h + g; fp16 copy for matmuls, fp32 copy
    (host range-reduced to [-pi, pi], offset folded in) for the angle path;
    MLP bias b1' = b1 - W1 @ off absorbs the offset for the MLP input
  - 3 MLP layers as fp16 block-pattern matmuls on PE (8 rows/cycle),
    tanh on ACT with per-partition bias; hidden activations stay fp16
  - PE transpose-mode matmuls accumulate x.T + corr.T into PSUM (the
    theta add is free)
  - sin/cos via ACT Sin on half-angles (s2 = sin(th/2), s4 = sin(th/4);
    cos th = 1-2*s2^2, sin th = 2*s2*(1-2*s4^2)) -- ACT Sin is only valid
    on [-pi, pi] and tanh+sin share one table set (silu_and_others, forced
    via a doctored table map so only one ACT table load happens)
  - chain of 6 (z-rot + const affine) steps as fp16 [128, 128] plane ops
    on DVE/GPSIMD, fk-derived scalars baked as immediates (program is
    recompiled if the non-joints inputs change; cached otherwise)
  - half 1's chain overlaps half 2's MLP; outputs DMA out per half

Measured: rel err ~8.8e-4 vs fp32 reference; ~36.5 us/core single-shot
(cost-model), ~25-48 us/iter measured on HW via an on-device For_i loop
(environment wall-clock noise is large).
"""

import os
import numpy as np

import concourse.bass as bass
import concourse.tile as tile
from concourse import bacc, mybir
from concourse import bass_utils

N_CORES = 8
CFG = {"t2_ov": "dve", "t4_tail": "pool", "q_tail": "dve", "sq4_tail": "dve", "zn_pool": "none", "m_pool": "none", "c1": 8, "comb_pool_ov": 0, "chb": 4, "hpb": 2, "mlpw": 1024}
B = 262144
BC = B // N_CORES            # 32768 rows per core
PLANE = BC // 128            # 256  (plane free size)
NCH = 16                     # transpose chunks of 128 cols

F16 = mybir.dt.float16
F32 = mybir.dt.float32
AF = mybir.ActivationFunctionType
OP = mybir.AluOpType

# ---- constants blob column map -------------------------------------------
C_BIAS1, C_BIAS2, C_BIAS3, C_HALFPI = 0, 1, 2, 3
# step-5 (first chain step) compound scalars
C_S5U1M, C_S5U1A, C_S5XM = 4, 5, 6
C_S5U3M, C_S5U3A, C_S5YM = 7, 8, 9
C_S5U5M, C_S5U5A, C_S5ZM = 10, 11, 12
# generic steps j=4..0: 5 scalars each starting at col 16: a, ca, sa, dsa, cad
def _CJ(j, k):
    return 16 + 5 * j + k
NCONST = 48


def _build_host_data(inputs):
    joints = np.asarray(inputs["joints"], np.float32)
    fk = np.asarray(inputs["fk_params"], np.float32)
    W1 = np.asarray(inputs["W1"], np.float32)
    b1 = np.asarray(inputs["b1"], np.float32)
    W2 = np.asarray(inputs["W2"], np.float32)
    b2 = np.asarray(inputs["b2"], np.float32)
    W3 = np.asarray(inputs["W3"], np.float32)
    b3 = np.asarray(inputs["b3"], np.float32)

    alpha, a, d, off = fk[:, 0], fk[:, 1], fk[:, 2], fk[:, 3]
    ca, sa = np.cos(alpha), np.sin(alpha)
    b1p = b1 - W1 @ off
    x_off = joints + off[None, :]          # [B, 7] fp32
    # exact host range-reduction for the angle path (Sin on ACT needs [-pi, pi];
    # device uses half-angle identities so th = x_red + corr stays in range)
    x_red = (np.remainder(x_off + np.pi, 2 * np.pi) - np.pi).astype(np.float32)

    # --- per-core feature-major images ---
    # batch row b = 256*p + 16*c + 8*h + g; partition q = 64*h + 8*k + g
    # x_img[q, 128*c + p] = x_off[b, k]
    def mkimg(src, dtype):
        out = []
        for core in range(N_CORES):
            jc = src[core * BC:(core + 1) * BC]            # [32768, 7]
            arr = jc.reshape(128, 16, 2, 8, 7)             # [p, c, h, g, k]
            arr = arr.transpose(2, 4, 3, 1, 0)             # [h, k, g, c, p]
            img = np.zeros((2, 8, 8, 16, 128), np.float32)
            img[:, :7] = arr
            out.append(np.ascontiguousarray(img.reshape(128, 2048)).astype(dtype))
        return out
    imgs32 = mkimg(x_red, np.float32)
    imgs16 = mkimg(x_off, np.float16)

    # --- block-pattern weights (fp16), packed into one [128, 304] blob ---
    blob = np.zeros((128, 304), np.float16)
    # L1: lhsT1[64h+8k+g, 15g+j] = W1[j, k]  (cols 0:120)
    for h in (0, 1):
        for k in range(7):
            for g in range(8):
                blob[64 * h + 8 * k + g, 15 * g:15 * g + 15] = W1[:, k]
    # L2: lhsT2[15g+i, 15g+j] = W2[j, i]  (cols 120:240)
    for g in range(8):
        blob[15 * g:15 * g + 15, 120 + 15 * g:120 + 15 * g + 15] = W2.T
    # L3: lhsT3[15g+i, 8k+g] = W3[k, i]  (cols 240:304)
    for g in range(8):
        for k in range(7):
            blob[15 * g:15 * g + 15, 240 + 8 * k + g] = W3[k, :]

    # --- constants blob [128, NCONST] fp32 ---
    consts = np.zeros((128, NCONST), np.float32)
    for g in range(8):
        for j in range(15):
            consts[15 * g + j, C_BIAS1] = b1p[j]
            consts[15 * g + j, C_BIAS2] = b2[j]
    for h in (0, 1):
        for k in range(7):
            for g in range(8):
                consts[64 * h + 8 * k + g, C_BIAS3] = b3[k]
    consts[:, C_HALFPI] = np.pi / 2

    t6 = np.array([a[6], -d[6] * sa[6], ca[6] * d[6]], np.float32)
    C1 = -sa[5] * t6[2] - d[5] * sa[5]
    C2 = ca[5] * t6[2] + ca[5] * d[5]
    consts[:, C_S5U1M] = a[6]
    consts[:, C_S5U1A] = a[5]
    consts[:, C_S5XM] = -t6[1]
    consts[:, C_S5U3M] = ca[5] * a[6]
    consts[:, C_S5U3A] = C1
    consts[:, C_S5YM] = ca[5] * t6[1]
    consts[:, C_S5U5M] = sa[5] * a[6]
    consts[:, C_S5U5A] = C2
    consts[:, C_S5ZM] = sa[5] * t6[1]
    for j in range(5):
        consts[:, _CJ(j, 0)] = a[j]
        consts[:, _CJ(j, 1)] = ca[j]
        consts[:, _CJ(j, 2)] = sa[j]
        consts[:, _CJ(j, 3)] = d[j] * sa[j]
        consts[:, _CJ(j, 4)] = ca[j] * d[j]

    id32 = np.ascontiguousarray(np.eye(128, dtype=np.float32))
    cblob = np.concatenate([
        blob.view(np.uint8).reshape(128, 608),
        consts.view(np.uint8).reshape(128, NCONST * 4),
        id32.view(np.uint8).reshape(128, 512),
    ], axis=1)
    return imgs32, imgs16, np.ascontiguousarray(cblob)


def _emit_program(nc, sc, reps=1, loop_n=0):
    dx16 = nc.dram_tensor("x16", [128, 2048], F16, kind="ExternalInput")
    dximg = nc.dram_tensor("ximg", [128, 2048], F32, kind="ExternalInput")
    # one constants blob: lhs f16 (608B) | consts f32 (NCONST*4) | id32 f32 (512B)
    CBYTES = 608 + NCONST * 4 + 512
    dcblob = nc.dram_tensor("cblob", [128, CBYTES], mybir.dt.uint8,
                            kind="ExternalInput")
    dout = nc.dram_tensor("out", [128, 768], F32, kind="ExternalOutput")

    from contextlib import ExitStack
    with tile.TileContext(nc) as tc, ExitStack() as ctx:
        cp = ctx.enter_context(tc.tile_pool(name="persist", bufs=1))
        hp = ctx.enter_context(tc.tile_pool(name="halfp", bufs=CFG["hpb"]))
        mlp_ps = ctx.enter_context(tc.tile_pool(name="mlpps", bufs=(4 if CFG["mlpw"] == 512 else 2), space="PSUM"))
        tp_ps = ctx.enter_context(tc.tile_pool(name="tpps", bufs=4, space="PSUM"))
        chp = ctx.enter_context(tc.tile_pool(name="chain", bufs=CFG["chb"]))

        cblob = cp.tile([128, CBYTES], mybir.dt.uint8, tag="cblob")
        lhs = cblob[:, 0:608].bitcast(F16)
        consts = cblob[:, 608:608 + NCONST * 4].bitcast(F32)
        id32 = cblob[:, 608 + NCONST * 4:CBYTES].bitcast(F32)

        # hoist the ACT table load under the input DMAs
        warm = cp.tile([128, 1], F32, tag="warm")
        nc.vector.memset(warm[:], 0.0)
        nc.scalar.activation(warm[:], warm[:], AF.Tanh, bias=0.0)

        def cv(col, parts=128):
            return consts[0:parts, col:col + 1]

        def mlp_half(x16, ximg, hf, C):
            """MLP + transpose + sincos for one batch-half (C chunks of 128).
            x16/ximg are this half's own [128, 128*C] tiles.
            Returns (CT, STt) [128, 128*C] fp16, plane j at [16*C*j : +16*C]."""
            nf = 128 * C
            PL = 16 * C
            h1 = hp.tile([128, 2 * nf], F16, tag="h1", name="h1")
            h2 = hp.tile([128, 2 * nf], F16, tag="h2", name="h2")
            corr = hp.tile([128, nf], F32, tag="corr", name="corr")
            CT = hp.tile([128, nf], F16, tag="CT", name="CT")
            STt = hp.tile([128, nf], F16, tag="ST", name="STt")
            S2 = hp.tile([128, nf], F16, tag="S2", name="S2")
            S4 = hp.tile([128, nf], F16, tag="S4", name="S4")
            SQ = hp.tile([128, nf], F16, tag="SQ", name="SQ")
            SQ4 = hp.tile([128, nf], F16, tag="SQ4", name="SQ4")

            slices = []
            o = 0
            while o < nf:
                w = min(CFG["mlpw"], nf - o)
                slices.append((o, w))
                o += w
            # L1: K=64 per h-half (gap layout), M=120; 1-bank psum tiles
            for h in (0, 1):
                for (o, w) in slices:
                    ps = mlp_ps.tile([128, CFG["mlpw"]], F32, tag="mlpps", name="ps")
                    for so in range(0, w, 512):
                        sw = min(512, w - so)
                        nc.tensor.matmul(
                            ps[0:120, so:so + sw],
                            lhs[64 * h:64 * h + 64, 0:120],
                            x16[64 * h:64 * h + 64, o + so:o + so + sw],
                            start=True, stop=True, tile_position=(64 * h, 0))
                    nc.scalar.activation(
                        h1[0:120, nf * h + o:nf * h + o + w],
                        ps[0:120, 0:w], AF.Tanh, bias=cv(C_BIAS1, 120))
            # L2
            for h in (0, 1):
                for (o, w) in slices:
                    ps = mlp_ps.tile([128, CFG["mlpw"]], F32, tag="mlpps", name="ps")
                    col = nf * h + o
                    for so in range(0, w, 512):
                        sw = min(512, w - so)
                        nc.tensor.matmul(
                            ps[0:120, so:so + sw],
                            lhs[0:120, 120:240],
                            h1[0:120, col + so:col + so + sw],
                            start=True, stop=True)
                    nc.scalar.activation(
                        h2[0:120, col:col + w],
                        ps[0:120, 0:w], AF.Tanh, bias=cv(C_BIAS2, 120))
            # L3: both h-halves stacked on psum partitions via col groups
            for (o, w) in slices:
                ps = mlp_ps.tile([128, CFG["mlpw"]], F32, tag="mlpps", name="ps")
                for h in (0, 1):
                    col = nf * h + o
                    for so in range(0, w, 512):
                        sw = min(512, w - so)
                        nc.tensor.matmul(
                            ps[64 * h:64 * h + 64, so:so + sw],
                            lhs[0:120, 240:304],
                            h2[0:120, col + so:col + so + sw],
                            start=True, stop=True, tile_position=(0, 64 * h))
                nc.scalar.activation(corr[:, o:o + w],
                                     ps[:, 0:w], AF.Tanh, bias=cv(C_BIAS3))

            # transposes (accumulate x.T + corr.T) + sincos + half-angle combine
            cbs = []
            o = 0
            while o < C:
                cw = min(4, C - o)
                cbs.append((o, cw))
                o += cw
            for (c0, cw) in cbs:
                ps2 = tp_ps.tile([128, 512], F32, tag="tpps", name="ps2")
                for cl in range(cw):
                    c = c0 + cl
                    nc.tensor.matmul(
                        ps2[:, 128 * cl:128 * cl + 128],
                        ximg[:, 128 * c:128 * c + 128], id32[:],
                        is_transpose=True, start=True, stop=False)
                    nc.tensor.matmul(
                        ps2[:, 128 * cl:128 * cl + 128],
                        corr[:, 128 * c:128 * c + 128], id32[:],
                        is_transpose=True, start=False, stop=True)
                # S2/S4 flat layout: ((k*C + c)*2 + h)*8 + g ; skip gap k=7
                in_v = ps2[:, 0:128 * cw].rearrange(
                    "p (c h k g) -> p c h k g", c=cw, h=2, k=8, g=8)[:, :, :, 0:6, :]
                s2_v = S2[:, :].rearrange(
                    "p (k c h g) -> p c h k g",
                    k=8, c=C, h=2, g=8)[:, c0:c0 + cw, :, 0:6, :]
                s4_v = S4[:, :].rearrange(
                    "p (k c h g) -> p c h k g",
                    k=8, c=C, h=2, g=8)[:, c0:c0 + cw, :, 0:6, :]
                nc.scalar.activation(s2_v, in_v, AF.Sin, bias=0.0, scale=0.5)
                nc.scalar.activation(s4_v, in_v, AF.Sin, bias=0.0, scale=0.25)
                # cos(th) = 1-2*s2^2 ; sin(th) = 2*s2*(1-2*s4^2)
                def cbv(tile_):
                    return tile_[:, :].rearrange(
                        "p (k c m) -> p k c m", k=8, c=C, m=16)[:, 0:6, c0:c0 + cw, :]
                allp = hf == 0 and CFG["comb_pool_ov"]
                e1 = nc.gpsimd if allp else nc.vector
                sq4eng = nc.gpsimd if (allp or (hf == 1 and CFG["sq4_tail"] == "pool")) else nc.vector
                meng = nc.gpsimd if (allp or CFG["m_pool"] in ("both", "tail" if hf == 1 else "ov")) else nc.vector
                nc.gpsimd.tensor_tensor(cbv(SQ), cbv(S2), cbv(S2), OP.mult)
                e1.tensor_scalar(cbv(CT), cbv(SQ), -2.0, 1.0, OP.mult, OP.add)
                sq4eng.tensor_tensor(cbv(SQ4), cbv(S4), cbv(S4), OP.mult)
                e1.tensor_scalar(cbv(S4), cbv(SQ4), -2.0, 1.0, OP.mult, OP.add)
                meng.tensor_tensor(cbv(SQ), cbv(S2), cbv(S4), OP.mult)
                e1.tensor_scalar(cbv(STt), cbv(SQ), 2.0, None, OP.mult)
            return CT, STt

        def chain_half(CT, STt, pack, hf, PL, m0):

            def ctj(j):
                return CT[:, PL * j:PL * j + PL]

            def stj(j):
                return STt[:, PL * j:PL * j + PL]

            def ch(tag):
                return chp.tile([128, PL], F16, tag=tag + str(hf), name=tag)

            u1 = ch("u1")
            nc.vector.tensor_scalar(u1, ctj(5), sc["s5u1m"], sc["s5u1a"], OP.mult, OP.add)
            x = ch("x")
            nc.vector.scalar_tensor_tensor(x, stj(5), sc["s5xm"], u1, OP.mult, OP.add)
            u3 = ch("u3")
            nc.vector.tensor_scalar(u3, stj(5), sc["s5u3m"], sc["s5u3a"], OP.mult, OP.add)
            y = ch("y")
            nc.vector.scalar_tensor_tensor(y, ctj(5), sc["s5ym"], u3, OP.mult, OP.add)
            u5 = ch("u5")
            nc.gpsimd.tensor_scalar(u5, stj(5), sc["s5u5m"], sc["s5u5a"], OP.mult, OP.add)
            z = ch("z")
            nc.vector.scalar_tensor_tensor(z, ctj(5), sc["s5zm"], u5, OP.mult, OP.add)

            tail = hf == 1
            for j in (4, 3, 2, 1, 0):
                last = j == 0
                a_j, ca_j, sa_j = sc[f"a{j}"], sc[f"ca{j}"], sc[f"sa{j}"]
                dsa_j, cad_j = sc[f"dsa{j}"], sc[f"cad{j}"]
                # q1 = z*sa + d*sa ; q2 = z*ca + ca*d  -- only need z, emit early
                q1 = ch("q1")
                q2 = ch("q2")
                if tail and CFG["q_tail"] == "act":
                    nc.scalar.activation(q1, z, AF.Identity,
                                         bias=cv(_CJ(j, 3)), scale=cv(_CJ(j, 2)))
                    nc.scalar.activation(q2, z, AF.Identity,
                                         bias=cv(_CJ(j, 4)), scale=cv(_CJ(j, 1)))
                else:
                    nc.vector.tensor_scalar(q1, z, sa_j, dsa_j, OP.mult, OP.add)
                    nc.vector.tensor_scalar(q2, z, ca_j, cad_j, OP.mult, OP.add)
                t1 = ch("t1")
                nc.vector.tensor_tensor(t1, x, ctj(j), OP.mult)
                t2 = ch("t2")
                t2eng = nc.gpsimd if (tail or CFG["t2_ov"] == "pool") else nc.vector
                t2eng.tensor_tensor(t2, y, stj(j), OP.mult)
                t3 = ch("t3")
                nc.vector.tensor_tensor(t3, x, stj(j), OP.mult)
                t4 = ch("t4")
                nc.gpsimd.tensor_tensor(t4, y, ctj(j), OP.mult) if tail else                     nc.vector.tensor_tensor(t4, y, ctj(j), OP.mult)
                yr = ch("yr")
                nc.vector.tensor_tensor(yr, t3, t4, OP.add)
                yn = pack[:, 256 + m0:256 + m0 + PL] if last else ch("y")
                zn = pack[:, 512 + m0:512 + m0 + PL] if last else ch("z")
                if tail and CFG["zn_pool"] == "actsplit":
                    u1t = ch("u1t")
                    nc.scalar.activation(u1t, yr, AF.Identity, bias=0.0,
                                         scale=cv(_CJ(j, 1)))
                    nc.vector.tensor_tensor(yn, u1t, q1, OP.subtract)
                    u2t = ch("u2t")
                    nc.scalar.activation(u2t, yr, AF.Identity, bias=0.0,
                                         scale=cv(_CJ(j, 2)))
                    nc.vector.tensor_tensor(zn, u2t, q2, OP.add)
                else:
                    nc.vector.scalar_tensor_tensor(yn, yr, ca_j, q1, OP.mult, OP.subtract)
                    nc.vector.scalar_tensor_tensor(zn, yr, sa_j, q2, OP.mult, OP.add)
                xn = pack[:, m0:m0 + PL] if last else ch("x")
                nc.vector.scalar_tensor_tensor(xn, t1, a_j, t2, OP.add, OP.subtract)
                x, y, z = xn, yn, zn

        # PE warm-up: dummy matmuls on a memset tile so the PE clock ramps
        # while the input DMAs are in flight.
        wm16 = cp.tile([128, 512], F16, tag="wm16")
        nc.gpsimd.memset(wm16[:], 0.0)
        wmps = mlp_ps.tile([128, CFG["mlpw"]], F32, tag="mlpps", name="wmps")
        for _w in range(3):
            nc.tensor.matmul(wmps[:, 0:512], wm16[0:64, 0:128],
                             wm16[0:64, :], start=True, stop=True)

        from contextlib import nullcontext
        loop_ctx = tc.For_i(0, loop_n, 1) if loop_n else nullcontext()
        first = True
        with loop_ctx:
          for _rep in range(reps):
              C1 = CFG["c1"]
              C2 = 16 - C1
              x16a = cp.tile([128, 128 * C1], F16, tag="x16a", name="x16a")
              x16b = cp.tile([128, 128 * C2], F16, tag="x16b", name="x16b")
              ximga = cp.tile([128, 128 * C1], F32, tag="ximga", name="ximga")
              ximgb = cp.tile([128, 128 * C2], F32, tag="ximgb", name="ximgb")
              pack = cp.tile([128, 768], F32, tag="pack", name="pack")
              nc.sync.dma_start(x16a[:], dx16.ap()[:, 0:128 * C1])
              if first:
                  nc.sync.dma_start(cblob[:], dcblob.ap())
                  first = False
              nc.sync.dma_start(x16b[:], dx16.ap()[:, 128 * C1:2048])
              nc.sync.dma_start(ximga[:], dximg.ap()[:, 0:128 * C1])
              nc.sync.dma_start(ximgb[:], dximg.ap()[:, 128 * C1:2048])
              CT0, ST0 = mlp_half(x16a, ximga, 0, C1)
              CT1, ST1 = mlp_half(x16b, ximgb, 1, C2)
              chain_half(CT0, ST0, pack, 0, 16 * C1, 0)
              chain_half(CT1, ST1, pack, 1, 16 * C2, 16 * C1)
              pv = pack[:, :].rearrange("p (c h) -> p c h", c=3, h=256)
              dv = dout.ap().rearrange("p (c h) -> p c h", c=3, h=256)
              nc.sync.dma_start(dv[:, :, 0:128], pv[:, :, 0:128])
              nc.sync.dma_start(dv[:, :, 128:256], pv[:, :, 128:256])


_PROG_CACHE = {}


def _baked_scalars(inputs):
    fk = np.asarray(inputs["fk_params"], np.float32)
    alpha, a, d = fk[:, 0], fk[:, 1], fk[:, 2]
    ca, sa = np.cos(alpha), np.sin(alpha)
    t6 = np.array([a[6], -d[6] * sa[6], ca[6] * d[6]], np.float32)
    sc = {
        "s5u1m": a[6], "s5u1a": a[5], "s5xm": -t6[1],
        "s5u3m": ca[5] * a[6], "s5u3a": -sa[5] * t6[2] - d[5] * sa[5],
        "s5ym": ca[5] * t6[1],
        "s5u5m": sa[5] * a[6], "s5u5a": ca[5] * t6[2] + ca[5] * d[5],
        "s5zm": sa[5] * t6[1],
    }
    for j in range(5):
        sc[f"a{j}"] = a[j]
        sc[f"ca{j}"] = ca[j]
        sc[f"sa{j}"] = sa[j]
        sc[f"dsa{j}"] = d[j] * sa[j]
        sc[f"cad{j}"] = ca[j] * d[j]
    return {k: float(np.float32(v)) for k, v in sc.items()}


def _get_program(inputs, reps=1, loop_n=0):
    sc = _baked_scalars(inputs)
    key = (tuple(sorted(sc.items())), reps, loop_n, tuple(sorted(CFG.items())))
    if key in _PROG_CACHE:
        return _PROG_CACHE[key]
    nc = bacc.Bacc("TRN2", target_bir_lowering=False, debug=False,
                   enable_asserts=False)
    _emit_program(nc, sc, reps=reps, loop_n=loop_n)

    # Force Tanh and Sin to resolve to the one table set containing both
    # (silu_and_others), so the kernel pays a single ACT table load.
    import concourse.bacc as bacc_mod
    from concourse.hw_specs import get_activation_tables
    orig_fn = bacc_mod.get_activation_tables
    tabs = get_activation_tables(nc.m.arch)
    trig = {AF.Tanh, AF.Sin}
    doctored = {
        name: (set(funcs) if name == "silu_and_others" else set(funcs) - trig)
        for name, funcs in tabs.items()
    }
    bacc_mod.get_activation_tables = lambda arch: doctored
    try:
        nc.compile()
    finally:
        bacc_mod.get_activation_tables = orig_fn

    _PROG_CACHE[key] = nc
    return nc


LAST_RESULTS = None  # BassKernelResults of the most recent run (for test.py)


def _host_in_maps(inputs):
    imgs32, imgs16, cblob = _build_host_data(inputs)
    in_maps = []
    for core in range(N_CORES):
        in_maps.append({
            "x16": imgs16[core],
            "ximg": imgs32[core],
            "cblob": cblob,
        })
    return in_maps


def _jit_runner(nc):
    import jax
    from jax.sharding import Mesh, PartitionSpec, NamedSharding
    from jax.experimental.shard_map import shard_map
    from concourse import bass2jax
    bass2jax.install_neuronx_cc_hook()

    partition_name = nc.partition_id_tensor.name if nc.partition_id_tensor else None
    in_names, out_names, out_avals = [], [], []
    for alloc in nc.m.functions[0].allocations:
        if not isinstance(alloc, mybir.MemoryLocationSet):
            continue
        name = alloc.memorylocations[0].name
        if alloc.kind == "ExternalInput":
            if name != partition_name:
                in_names.append(name)
        elif alloc.kind == "ExternalOutput":
            out_names.append(name)
            out_avals.append(jax.core.ShapedArray(
                tuple(alloc.tensor_shape), mybir.dt.np(alloc.dtype)))
    all_in = in_names + out_names + ([partition_name] if partition_name else [])
    devices = jax.devices()[:N_CORES]
    mesh = Mesh(np.asarray(devices), ("core",))
    sh = NamedSharding(mesh, PartitionSpec("core"))

    def _body(*args):
        ops = list(args)
        if partition_name:
            ops.append(bass2jax.partition_id_tensor())
        outs = bass2jax._bass_exec_p.bind(
            *ops, out_avals=tuple(out_avals), in_names=tuple(all_in),
            out_names=tuple(out_names), lowering_input_output_aliases=(),
            sim_require_finite=True, sim_require_nnan=True, nc=nc)
        return tuple(outs)

    specs = (PartitionSpec("core"),) * (len(in_names) + len(out_names))
    ospec = (PartitionSpec("core"),) * len(out_names)
    f = jax.jit(shard_map(_body, mesh=mesh, in_specs=specs, out_specs=ospec,
                          check_rep=False))
    return f, in_names, out_avals, sh


def time_on_hw(inputs, n_lo=16, n_hi=256, iters=10):
    """Per-kernel HW time via an on-device For_i loop: slope of min wall
    between trip counts (includes ~2us loop back-edge per iteration)."""
    import time as _time
    import jax
    in_maps = _host_in_maps(inputs)
    mins = {}
    for loop_n in (n_lo, n_hi):
        nc = _get_program(inputs, loop_n=loop_n)
        f, in_names, out_avals, sh = _jit_runner(nc)
        cat = lambda n: np.concatenate(
            [np.asarray(in_maps[c][n]) for c in range(N_CORES)], axis=0)
        ci = [jax.device_put(cat(n), sh) for n in in_names]
        cz = [jax.device_put(
            np.zeros((N_CORES * a.shape[0], *a.shape[1:]), a.dtype), sh)
            for a in out_avals]
        jax.block_until_ready(f(*ci, *cz))
        best = float("inf")
        for _ in range(iters):
            t0 = _time.perf_counter()
            jax.block_until_ready(f(*ci, *cz))
            best = min(best, _time.perf_counter() - t0)
        mins[loop_n] = best
        print(f"[hw timing] loop_n={loop_n}: min wall {best*1e3:.2f} ms")
    slope_ns = (mins[n_hi] - mins[n_lo]) / (n_hi - n_lo) * 1e9
    print(f"[hw timing] -> {slope_ns:.0f} ns/kernel (incl ~2us loop overhead)")
    return slope_ns


def kernel(**inputs):
    global LAST_RESULTS
    j = np.asarray(inputs["joints"])
    assert j.shape == (B, 7), f"kernel hardcodes joints shape {(B, 7)}, got {j.shape}"
    nc = _get_program(inputs)
    in_maps = _host_in_maps(inputs)
    res = bass_utils.run_bass_kernel_spmd(nc, in_maps, core_ids=list(range(N_CORES)))
    LAST_RESULTS = res

    out = np.empty((B, 3), np.float32)
    for core in range(N_CORES):
        p = res.results[core]["out"]                   # [128, 768]
        # pack cols: [px(256) | py(256) | pz(256)], b_local = 256*p + m
        oc = p.reshape(128, 3, 256).transpose(0, 2, 1).reshape(BC, 3)
        out[core * BC:(core + 1) * BC] = oc
    return out



# revision 30
# speedup vs baseline: 2.1707x; 2.1707x over previous
"""Trainium2 Bass kernel for batched FK chain with tanh-MLP joint correction.

Math: per batch row,
    corr = tanh MLP_{7-15-15-7}(joints);  th = joints + off + corr
    M_j = DH(alpha_j, a_j, d_j, th_j);    out = (M_0 @ ... @ M_6)[:3, 3]
Factorization: M_j = A_j @ Rz(th_j) with A_j constant, and col 3 of M_6 is
constant, so the chain is 6 steps of (z-rotation + constant affine) on a
3-vector.

Distribution: pure data parallel, batch/8 = 32768 rows per NeuronCore.

Per-core pipeline (two batch halves, pipelined against each other, all fp16):
  - host packs feature-major fp16 images [128, 2048]: partition q = 64h+8k+g
    (16 batch groups x 7 features, 8 gap rows), free n = 128c+p,
    batch row b = 256p + 16c + 8h + g; x16 = joints+offset (MLP input,
    offset folded via b1' = b1 - W1 @ off), xr16 = range-reduced copy
    (angle path; Sin on ACT is only valid on [-pi, pi])
  - 3 MLP layers as fp16 block-pattern matmuls on PE (8 rows/cycle),
    tanh on ACT with per-partition bias; hidden activations and corr fp16
  - PE transpose-mode fp16 matmuls accumulate xr.T early (under the MLP)
    and corr.T after L3 into PSUM (the theta add is free)
  - sin/cos via ACT Sin on half-angles: s2 = sin(th/2), s4 = sin(th/4);
    cos th = 1-2*s2^2, sin th = s2*(2-4*s4^2) -- tanh+sin+identity share
    one table set (silu_and_others, forced via a doctored table map)
  - chain of 6 (z-rot + const affine) steps as fp16 [128, 128] plane ops
    split across DVE/GPSIMD/ACT per a tuned engine map; fk-derived scalars
    baked as immediates (program recompiled if non-joints inputs change)
  - half 1's chain overlaps half 2's MLP; per-half output DMA
"""

import numpy as np

import concourse.bass as bass
import concourse.tile as tile
from concourse import bacc, mybir
from concourse import bass_utils

N_CORES = 8
B = 262144
BC = B // N_CORES            # 32768 rows per core
C1 = 8                       # 128-col chunks in half 0 (psum: keep 8)
C2 = 16 - C1                 # chunks in half 1

F16 = mybir.dt.float16
F32 = mybir.dt.float32
AF = mybir.ActivationFunctionType
OP = mybir.AluOpType

C_BIAS1, C_BIAS2, C_BIAS3 = 0, 1, 2
# step-5 compound scalars (m, a) pairs, then generic steps j=0..4
C_ZERO = 3
C_M4, C_P2, C_M2, C_P1 = 10, 11, 12, 13
C_S5U1M, C_S5U1A = 4, 5
C_S5U3M, C_S5U3A = 6, 7
C_S5U5M, C_S5U5A = 8, 9
def _CJ(j, k):
    # k: 0=a, 1=ca, 2=sa, 3=dsa, 4=cad
    return 16 + 5 * j + k
NCONST = 48

# Engine codes: v=DVE, p=GPSIMD(Pool), a=ACT (ts-type ops only).
# Chain ops run once on merged [128, 256] planes; q/u ops may ride ACT
# (Identity with consts-blob scale/bias). scalar_tensor_tensor is DVE-only.
CFG = {
    # chain engine maps: merged mode uses ["ch"][None]; split mode uses
    # per-half maps ["ch"][0]/["ch"][1] (h0 chain hides under h1's MLP).
    "split_chain": True,
    "c1": 8,                  # chunks in half 0 (16 - c1 in half 1)
    "l1_split0": True,        # split h0's L1 tanh into [*,512] pieces
    "block_sins": False,      # sins per 4-chunk block (pipelines vs PE)
    "chain0_early": False,    # emit h0's chain before h1's MLP
    "ch": {
        None: {"t1": "p", "t2": "p", "t3": "v", "t4": "v", "yr": "v",
               "q1": "a", "q2": "a", "u1": "a", "u3": "a", "u5": "a",
               "xn": "v", "yn": "v", "zn": "v",
               "x5": "v", "y5": "v", "z5": "v"},
        0: {"t1": "p", "t2": "p", "t3": "v", "t4": "v", "yr": "v",
            "q1": "p", "q2": "p", "u1": "p", "u3": "p", "u5": "p",
            "xn": "v", "yn": "v", "zn": "v",
            "x5": "v", "y5": "v", "z5": "v"},
        1: {"t1": "p", "t2": "v", "t3": "v", "t4": "v", "yr": "v",
            "q1": "p", "q2": "p", "u1": "a", "u3": "a", "u5": "a",
            "xn": "v", "yn": "v", "zn": "v",
            "x5": "v", "y5": "v", "z5": "v"},
    },
    # per-half combine engine map + plane-group splits (descending k so
    # the chain's first-needed planes are ready soonest)
    "cb": {0: {"SQ": "v", "CT": "v", "SQ4": "v", "C2": "v", "ST": "v"},
           1: {"SQ": "v", "CT": "v", "SQ4": "v", "C2": "v", "ST": "v"}},
    "cb_groups": {0: [(0, 6)], 1: [(0, 6)]},
}


def _build_host_data(inputs):
    joints = np.asarray(inputs["joints"], np.float32)
    fk = np.asarray(inputs["fk_params"], np.float32)
    W1 = np.asarray(inputs["W1"], np.float32)
    b1 = np.asarray(inputs["b1"], np.float32)
    W2 = np.asarray(inputs["W2"], np.float32)
    b2 = np.asarray(inputs["b2"], np.float32)
    W3 = np.asarray(inputs["W3"], np.float32)
    b3 = np.asarray(inputs["b3"], np.float32)

    off = fk[:, 3]
    b1p = b1 - W1 @ off
    x_off = joints + off[None, :]          # [B, 7] fp32
    # exact host range-reduction for the angle path (Sin on ACT needs
    # [-pi, pi]; device uses half-angle identities so th = x_red + corr
    # stays in range)
    x_red = (np.remainder(x_off + np.pi, 2 * np.pi) - np.pi).astype(np.float32)

    # --- per-core feature-major images ---
    # batch row b = 256*p + 16*c + 8*h + g; partition q = 64*h + 8*k + g
    # img[q, 128*c + p] = src[b, k]
    def mkimg(src, dtype):
        out = []
        for core in range(N_CORES):
            jc = src[core * BC:(core + 1) * BC]            # [32768, 7]
            arr = jc.reshape(128, 16, 2, 8, 7)             # [p, c, h, g, k]
            arr = arr.transpose(2, 4, 3, 1, 0)             # [h, k, g, c, p]
            img = np.zeros((2, 8, 8, 16, 128), np.float32)
            img[:, :7] = arr
            out.append(np.ascontiguousarray(
                img.reshape(128, 2048)).astype(dtype))
        return out
    imgs16 = mkimg(x_off, np.float16)
    imgs32 = mkimg(x_red, np.float32)

    # --- block-pattern weights (fp16), packed into one [128, 304] blob ---
    blob = np.zeros((128, 304), np.float16)
    # L1: lhsT1[64h+8k+g, 15g+j] = W1[j, k]  (cols 0:120)
    for h in (0, 1):
        for k in range(7):
            for g in range(8):
                blob[64 * h + 8 * k + g, 15 * g:15 * g + 15] = W1[:, k]
    # L2: lhsT2[15g+i, 15g+j] = W2[j, i]  (cols 120:240)
    for g in range(8):
        blob[15 * g:15 * g + 15, 120 + 15 * g:120 + 15 * g + 15] = W2.T
    # L3: lhsT3[15g+i, 8k+g] = W3[k, i]  (cols 240:304)
    for g in range(8):
        for k in range(7):
            blob[15 * g:15 * g + 15, 240 + 8 * k + g] = W3[k, :]

    # --- constants blob [128, NCONST] fp32 (per-partition MLP biases) ---
    consts = np.zeros((128, NCONST), np.float32)
    for g in range(8):
        for j in range(15):
            consts[15 * g + j, C_BIAS1] = b1p[j]
            consts[15 * g + j, C_BIAS2] = b2[j]
    for h in (0, 1):
        for k in range(7):
            for g in range(8):
                consts[64 * h + 8 * k + g, C_BIAS3] = b3[k]

    alpha, a, d = fk[:, 0], fk[:, 1], fk[:, 2]
    ca, sa = np.cos(alpha), np.sin(alpha)
    t6 = np.array([a[6], -d[6] * sa[6], ca[6] * d[6]], np.float32)
    consts[:, C_M4] = -4.0
    consts[:, C_P2] = 2.0
    consts[:, C_M2] = -2.0
    consts[:, C_P1] = 1.0
    consts[:, C_S5U1M] = a[6]
    consts[:, C_S5U1A] = a[5]
    consts[:, C_S5U3M] = ca[5] * a[6]
    consts[:, C_S5U3A] = -sa[5] * t6[2] - d[5] * sa[5]
    consts[:, C_S5U5M] = sa[5] * a[6]
    consts[:, C_S5U5A] = ca[5] * t6[2] + ca[5] * d[5]
    for j in range(5):
        consts[:, _CJ(j, 0)] = a[j]
        consts[:, _CJ(j, 1)] = ca[j]
        consts[:, _CJ(j, 2)] = sa[j]
        consts[:, _CJ(j, 3)] = d[j] * sa[j]
        consts[:, _CJ(j, 4)] = ca[j] * d[j]

    id32 = np.ascontiguousarray(np.eye(128, dtype=np.float32))
    cblob = np.concatenate([
        blob.view(np.uint8).reshape(128, 608),
        consts.view(np.uint8).reshape(128, NCONST * 4),
        id32.view(np.uint8).reshape(128, 512),
    ], axis=1)
    return imgs16, imgs32, np.ascontiguousarray(cblob)


def _emit_program(nc, sc, reps=1, loop_n=0):
    dx16 = nc.dram_tensor("x16", [128, 2048], F16, kind="ExternalInput")
    dximg = nc.dram_tensor("ximg", [128, 2048], F32, kind="ExternalInput")
    CBYTES = 608 + NCONST * 4 + 512
    dcblob = nc.dram_tensor("cblob", [128, CBYTES], mybir.dt.uint8,
                            kind="ExternalInput")
    dout = nc.dram_tensor("out", [128, 768], F32, kind="ExternalOutput")
    from contextlib import ExitStack, nullcontext
    with tile.TileContext(nc) as tc, ExitStack() as ctx:
        cp = ctx.enter_context(tc.tile_pool(name="persist", bufs=1))
        ip = ctx.enter_context(tc.tile_pool(name="inp", bufs=2))
        hp = ctx.enter_context(tc.tile_pool(name="halfp", bufs=2))
        mlp_ps = ctx.enter_context(
            tc.tile_pool(name="mlpps", bufs=2, space="PSUM"))
        tp_ps = ctx.enter_context(
            tc.tile_pool(name="tpps", bufs=2, space="PSUM"))
        chp = ctx.enter_context(tc.tile_pool(name="chain", bufs=4))

        cblob = cp.tile([128, CBYTES], mybir.dt.uint8, tag="cblob")
        lhs = cblob[:, 0:608].bitcast(F16)
        consts = cblob[:, 608:608 + NCONST * 4].bitcast(F32)
        id32 = cblob[:, 608 + NCONST * 4:CBYTES].bitcast(F32)

        # hoist the ACT table load under the input DMAs
        warm = cp.tile([128, 1], F32, tag="warm")
        nc.vector.memset(warm[:], 0.0)
        nc.scalar.activation(warm[:], warm[:], AF.Tanh, bias=0.0)

        def cv(col, parts=128):
            return consts[0:parts, col:col + 1]

        def eng(code):
            return nc.vector if code == "v" else nc.gpsimd

        def sq_op(code, out, in_):
            """out = in_^2 on DVE/Pool tt or ACT Square."""
            if code == "a":
                nc.scalar.activation(out, in_, AF.Square, bias=0.0)
            else:
                eng(code).tensor_tensor(out, in_, in_, OP.mult)

        def ts_op(code, out, in_, mul, add, mcol=None, acol=None):
            """out = in_*mul + add on DVE/Pool ts or ACT identity.
            For the ACT path, mcol/acol are consts-blob columns holding the
            same scalars (ACT bias/scale must be APs)."""
            if code == "a":
                nc.scalar.activation(out, in_, AF.Identity,
                                     bias=cv(acol), scale=cv(mcol))
            else:
                eng(code).tensor_scalar(out, in_, float(mul), float(add),
                                        OP.mult, OP.add)

        # Merged trig tiles span both halves: plane k at cols
        # [256k, 256k+256), col = 256k + 16cg + 8h + g, cg = global chunk.
        def halfview(t, c0, C):
            return t.rearrange("p (k cg h g) -> p k cg h g",
                               k=6, cg=16, h=2, g=8)[:, :, c0:c0 + C]

        def mlp_half(x16h, xrh, hf, C, l1_split=False, block_sins=None):
            """MLP + transposes for one half; returns the theta psum."""
            NF = 128 * C
            half = 512 * ((NF // 2) // 512) if NF // 2 >= 512 else NF
            ps2 = tp_ps.tile([128, NF], F32, tag="tpps", name="ps2")
            h1 = hp.tile([128, 2 * NF], F16, tag="h1", name="h1")
            h2 = hp.tile([128, 2 * NF], F16, tag="h2", name="h2")
            corr = hp.tile([128, NF], F32, tag="corr", name="corr")

            def mm_slices(n0, n1):
                out = []
                o = n0
                while o < n1:
                    w = min(512, n1 - o)
                    out.append((o, w))
                    o += w
                return out

            # L1: K=64 per h-half (gap layout), M=120
            for h in (0, 1):
                ps = mlp_ps.tile([128, NF], F32, tag="mlpps", name="ps")
                pieces = [(0, half), (half, NF)] if l1_split else [(0, NF)]
                for (p0, p1) in pieces:
                    if p0 == p1:
                        continue
                    for (o, w) in mm_slices(p0, p1):
                        nc.tensor.matmul(
                            ps[0:120, o:o + w],
                            lhs[64 * h:64 * h + 64, 0:120],
                            x16h[64 * h:64 * h + 64, o:o + w],
                            start=True, stop=True, tile_position=(64 * h, 0))
                    nc.scalar.activation(
                        h1[0:120, NF * h + p0:NF * h + p1],
                        ps[0:120, p0:p1], AF.Tanh, bias=cv(C_BIAS1, 120))
            # L2
            for h in (0, 1):
                ps = mlp_ps.tile([128, NF], F32, tag="mlpps", name="ps")
                for (o, w) in mm_slices(0, NF):
                    nc.tensor.matmul(
                        ps[0:120, o:o + w],
                        lhs[0:120, 120:240],
                        h1[0:120, NF * h + o:NF * h + o + w],
                        start=True, stop=True)
                nc.scalar.activation(
                    h2[0:120, NF * h:NF * h + NF],
                    ps[0:120, :], AF.Tanh, bias=cv(C_BIAS2, 120))
            # L3: both h-halves stacked on psum partitions via col groups;
            # tanh split in two; second L3 matmul group is emitted before the
            # first transpose batch so ACT L3a/L3b run back-to-back
            ps = mlp_ps.tile([128, NF], F32, tag="mlpps", name="ps")

            def l3mm(n0, n1):
                for h in (0, 1):
                    for (o, w) in mm_slices(n0, n1):
                        nc.tensor.matmul(
                            ps[64 * h:64 * h + 64, o:o + w],
                            lhs[0:120, 240:304],
                            h2[0:120, NF * h + o:NF * h + o + w],
                            start=True, stop=True, tile_position=(0, 64 * h))

            def tp(ca, cb):
                # transpose pairs (xr + corr accumulate) -> th = xr + corr
                for c in range(ca, cb):
                    nc.tensor.matmul(
                        ps2[:, 128 * c:128 * c + 128],
                        xrh[:, 128 * c:128 * c + 128], id32[:],
                        is_transpose=True, start=True, stop=False)
                    nc.tensor.matmul(
                        ps2[:, 128 * c:128 * c + 128],
                        corr[:, 128 * c:128 * c + 128], id32[:],
                        is_transpose=True, start=False, stop=True)
            l3mm(0, half)
            nc.scalar.activation(corr[:, 0:half], ps[:, 0:half],
                                 AF.Tanh, bias=cv(C_BIAS3))
            l3mm(half, NF)
            tp(0, half // 128)
            nc.scalar.activation(corr[:, half:NF], ps[:, half:NF],
                                 AF.Tanh, bias=cv(C_BIAS3))
            if block_sins:
                block_sins(ps2, 0, half // 128)
            tp(half // 128, C)
            if block_sins:
                block_sins(ps2, half // 128, C)
            return ps2

        def sin_part(hf, ps2, c0, C, S2, S4, CT, ST, SQ, CB2, b0=0, b1=None):
            """Sins + combine for chunk sub-range [c0+b0, c0+b1)."""
            em = CFG["cb"][hf]
            if b1 is None:
                b1 = C
            # sins: s2 = sin(th/2), s4 = sin(th/4); into merged plane layout
            in_all = ps2[:, :].rearrange(
                "p (c h k g) -> p k c h g", c=C, h=2, k=8, g=8)[:, 0:6,
                                                                b0:b1]
            nc.scalar.activation(halfview(S2, c0 + b0, b1 - b0), in_all,
                                 AF.Sin, bias=0.0, scale=0.5)
            nc.scalar.activation(halfview(S4, c0 + b0, b1 - b0), in_all,
                                 AF.Sin, bias=0.0, scale=0.25)
            # combine: CT = 1-2*s2^2 ; ST = s2*(2-4*s4^2).
            # The s4 branch (SQ4/C2) may ride ACT (Square + Identity) --
            # after the last sins ACT is otherwise idle, freeing DVE.
            groups = CFG["cb_groups"][hf]
            for (k0, k1) in groups:
                def hv(t):
                    return halfview(t, c0 + b0, b1 - b0)[:, k0:k1]
                s2v, s4v, sqv, c2v = hv(S2), hv(S4), hv(SQ), hv(CB2)
                ctv, stv = hv(CT), hv(ST)
                # SQ (sin2-dep) first; the s4 branch may ride ACT
                if em["SQ4"] == "a":
                    sq_op("a", c2v, s4v)
                    ts_op(em["C2"], c2v, c2v, -4.0, 2.0, C_M4, C_P2)
                    sq_op(em["SQ"], sqv, s2v)
                    ts_op(em["CT"], ctv, sqv, -2.0, 1.0, C_M2, C_P1)
                else:
                    sq_op(em["SQ"], sqv, s2v)
                    ts_op(em["CT"], ctv, sqv, -2.0, 1.0, C_M2, C_P1)
                    sq_op(em["SQ4"], c2v, s4v)
                    ts_op(em["C2"], c2v, c2v, -4.0, 2.0, C_M4, C_P2)
                eng(em["ST"]).tensor_tensor(stv, s2v, c2v, OP.mult)

        def chain_run(CT, ST, pack, hf=None, c0=0, C=16, row_dma=None):
            """One FK chain pass over chunk range [c0, c0+C).
            row_dma(row, sl): called right after the final x/y/z write so the
            output DMAs overlap the last step (row: 0=x,1=y,2=z)."""
            em = CFG["ch"][hf]
            W = 16 * C
            off = 16 * c0

            def ctj(j):
                return CT[:, 256 * j + off:256 * j + off + W]

            def stj(j):
                return ST[:, 256 * j + off:256 * j + off + W]

            def ch(tag):
                return chp.tile([128, W], F16, tag=tag + str(hf), name=tag)

            # step 5 (innermost): affine in (c5, s5)
            u1 = ch("u1")
            ts_op(em["u1"], u1, ctj(5), sc["s5u1m"], sc["s5u1a"],
                  C_S5U1M, C_S5U1A)
            u3 = ch("u3")
            ts_op(em["u3"], u3, stj(5), sc["s5u3m"], sc["s5u3a"],
                  C_S5U3M, C_S5U3A)
            u5 = ch("u5")
            ts_op(em["u5"], u5, stj(5), sc["s5u5m"], sc["s5u5a"],
                  C_S5U5M, C_S5U5A)
            x = ch("x")
            eng(em["x5"]).scalar_tensor_tensor(
                x, stj(5), sc["s5xm"], u1, OP.mult, OP.add)
            y = ch("y")
            eng(em["y5"]).scalar_tensor_tensor(
                y, ctj(5), sc["s5ym"], u3, OP.mult, OP.add)
            z = ch("z")
            eng(em["z5"]).scalar_tensor_tensor(
                z, ctj(5), sc["s5zm"], u5, OP.mult, OP.add)

            for j in (4, 3, 2, 1, 0):
                last = j == 0
                a_j, ca_j, sa_j = sc[f"a{j}"], sc[f"ca{j}"], sc[f"sa{j}"]
                dsa_j, cad_j = sc[f"dsa{j}"], sc[f"cad{j}"]
                # q's early (dep: z only); t1/t2 on Pool feed xn late
                q1 = ch("q1")
                ts_op(em["q1"], q1, z, sa_j, dsa_j, _CJ(j, 2), _CJ(j, 3))
                q2 = ch("q2")
                ts_op(em["q2"], q2, z, ca_j, cad_j, _CJ(j, 1), _CJ(j, 4))
                t1 = ch("t1")
                eng(em["t1"]).tensor_tensor(t1, x, ctj(j), OP.mult)
                t2 = ch("t2")
                eng(em["t2"]).tensor_tensor(t2, y, stj(j), OP.mult)
                t3 = ch("t3")
                eng(em["t3"]).tensor_tensor(t3, x, stj(j), OP.mult)
                t4 = ch("t4")
                eng(em["t4"]).tensor_tensor(t4, y, ctj(j), OP.mult)
                yr = ch("yr")
                eng(em["yr"]).tensor_tensor(yr, t3, t4, OP.add)
                yn = pack[:, 256 + off:256 + off + W] if last else ch("y")
                eng(em["yn"]).scalar_tensor_tensor(
                    yn, yr, ca_j, q1, OP.mult, OP.subtract)
                if last and row_dma:
                    row_dma(1)
                zn = pack[:, 512 + off:512 + off + W] if last else ch("z")
                eng(em["zn"]).scalar_tensor_tensor(
                    zn, yr, sa_j, q2, OP.mult, OP.add)
                if last and row_dma:
                    row_dma(2)
                xn = pack[:, off:off + W] if last else ch("x")
                eng(em["xn"]).scalar_tensor_tensor(
                    xn, t1, a_j, t2, OP.add, OP.subtract)
                if last and row_dma:
                    row_dma(0)
                x, y, z = xn, yn, zn

        # PE warm-up: dummy matmuls on a memset tile so the PE clock ramps
        # while the input DMAs are in flight.
        wm16 = cp.tile([128, 512], F16, tag="wm16")
        nc.vector.memset(wm16[:], 0.0)
        wmps = mlp_ps.tile([128, 1024], F32, tag="mlpps", name="wmps")
        for _w in range(3):
            nc.tensor.matmul(wmps[:, 0:512], wm16[0:64, 0:128],
                             wm16[0:64, :], start=True, stop=True)

        loop_ctx = tc.For_i(0, loop_n, 1) if loop_n else nullcontext()
        first = True
        with loop_ctx:
          for _rep in range(reps):
              NF1, NF2 = 128 * C1, 128 * C2
              x16a = ip.tile([128, NF1], F16, tag="x16a", name="x16a")
              x16b = ip.tile([128, NF2], F16, tag="x16b", name="x16b")
              xra = ip.tile([128, NF1], F32, tag="xra", name="xra")
              xrb = ip.tile([128, NF2], F32, tag="xrb", name="xrb")
              pack = ip.tile([128, 768], F32, tag="pack", name="pack")
              S2 = ip.tile([128, 1536], F16, tag="S2", name="S2")
              S4 = ip.tile([128, 1536], F16, tag="S4", name="S4")
              CT = ip.tile([128, 1536], F16, tag="CT", name="CT")
              ST = ip.tile([128, 1536], F16, tag="ST", name="ST")
              SQ = ip.tile([128, 1536], F16, tag="SQ", name="SQ")
              CB2 = ip.tile([128, 1536], F16, tag="CB2", name="CB2")
              # first DMA split so h0's first L1 piece starts sooner
              nc.sync.dma_start(x16a[:, 0:512], dx16.ap()[:, 0:512])
              if first:
                  nc.sync.dma_start(cblob[:], dcblob.ap())
                  first = False
              nc.sync.dma_start(x16a[:, 512:NF1], dx16.ap()[:, 512:NF1])
              nc.sync.dma_start(x16b[:], dx16.ap()[:, NF1:2048])
              nc.sync.dma_start(xra[:], dximg.ap()[:, 0:NF1])
              nc.sync.dma_start(xrb[:], dximg.ap()[:, NF1:2048])
              pv = pack[:, :].rearrange("p (c h) -> p c h", c=3, h=256)
              dv = dout.ap().rearrange("p (c h) -> p c h", c=3, h=256)
              def mk_bs(hf, c0, C):
                  def bs(ps2, b0, b1):
                      sin_part(hf, ps2, c0, C, S2, S4, CT, ST, SQ, CB2,
                               b0, b1)
                  return bs
              BS = CFG["block_sins"]
              ps2a = mlp_half(x16a, xra, 0, C1, l1_split=CFG["l1_split0"],
                              block_sins=mk_bs(0, 0, C1) if BS else None)
              if not BS:
                  sin_part(0, ps2a, 0, C1, S2, S4, CT, ST, SQ, CB2)
              if CFG["split_chain"]:
                  W1 = 16 * C1
                  def h1_mlp_sins():
                      ps2b = mlp_half(x16b, xrb, 1, C2,
                                      block_sins=mk_bs(1, C1, C2) if BS
                                      else None)
                      if not BS:
                          sin_part(1, ps2b, C1, C2, S2, S4, CT, ST, SQ, CB2)
                  def row_dma(row):
                      nc.sync.dma_start(dv[:, row:row + 1, W1:256],
                                        pv[:, row:row + 1, W1:256])
                  if CFG["chain0_early"]:
                      chain_run(CT, ST, pack, 0, 0, C1)
                      nc.sync.dma_start(dv[:, :, 0:W1], pv[:, :, 0:W1])
                      h1_mlp_sins()
                  else:
                      h1_mlp_sins()
                      chain_run(CT, ST, pack, 0, 0, C1)
                      nc.sync.dma_start(dv[:, :, 0:W1], pv[:, :, 0:W1])
                  chain_run(CT, ST, pack, 1, C1, C2, row_dma=row_dma)
              else:
                  ps2b = mlp_half(x16b, xrb, 1, C2,
                                  block_sins=mk_bs(1, C1, C2) if BS else None)
                  if not BS:
                      sin_part(1, ps2b, C1, C2, S2, S4, CT, ST, SQ, CB2)
                  chain_run(CT, ST, pack)
                  nc.sync.dma_start(dv[:, :, 0:128], pv[:, :, 0:128])
                  nc.sync.dma_start(dv[:, :, 128:256], pv[:, :, 128:256])


_PROG_CACHE = {}


def _baked_scalars(inputs):
    fk = np.asarray(inputs["fk_params"], np.float32)
    alpha, a, d = fk[:, 0], fk[:, 1], fk[:, 2]
    ca, sa = np.cos(alpha), np.sin(alpha)
    t6 = np.array([a[6], -d[6] * sa[6], ca[6] * d[6]], np.float32)
    sc = {
        "s5u1m": a[6], "s5u1a": a[5], "s5xm": -t6[1],
        "s5u3m": ca[5] * a[6], "s5u3a": -sa[5] * t6[2] - d[5] * sa[5],
        "s5ym": ca[5] * t6[1],
        "s5u5m": sa[5] * a[6], "s5u5a": ca[5] * t6[2] + ca[5] * d[5],
        "s5zm": sa[5] * t6[1],
    }
    for j in range(5):
        sc[f"a{j}"] = a[j]
        sc[f"ca{j}"] = ca[j]
        sc[f"sa{j}"] = sa[j]
        sc[f"dsa{j}"] = d[j] * sa[j]
        sc[f"cad{j}"] = ca[j] * d[j]
    return {k: float(np.float32(v)) for k, v in sc.items()}


def _cfg_key():
    def frz(v):
        if isinstance(v, dict):
            return tuple(sorted(((str(k), frz(x)) for k, x in v.items())))
        if isinstance(v, list):
            return tuple(v)
        return v
    return frz(CFG)


def _get_program(inputs, reps=1, loop_n=0):
    sc = _baked_scalars(inputs)
    key = (tuple(sorted(sc.items())), reps, loop_n, _cfg_key())
    if key in _PROG_CACHE:
        return _PROG_CACHE[key]
    nc = bacc.Bacc("TRN2", target_bir_lowering=False, debug=False,
                   enable_asserts=False)
    _emit_program(nc, sc, reps=reps, loop_n=loop_n)

    # Force Tanh/Sin (and Identity) to resolve to the one table set
    # containing all of them (silu_and_others), so the kernel pays a single
    # ACT table load.
    import concourse.bacc as bacc_mod
    from concourse.hw_specs import get_activation_tables
    orig_fn = bacc_mod.get_activation_tables
    tabs = get_activation_tables(nc.m.arch)
    trig = {AF.Tanh, AF.Sin, AF.Identity}
    doctored = {
        name: (set(funcs) if name == "silu_and_others" else set(funcs) - trig)
        for name, funcs in tabs.items()
    }
    bacc_mod.get_activation_tables = lambda arch: doctored
    try:
        nc.compile()
    finally:
        bacc_mod.get_activation_tables = orig_fn

    _PROG_CACHE[key] = nc
    return nc


LAST_RESULTS = None  # BassKernelResults of the most recent run (for test.py)


def _host_in_maps(inputs):
    imgs16, imgs32, cblob = _build_host_data(inputs)
    in_maps = []
    for core in range(N_CORES):
        in_maps.append({
            "x16": imgs16[core],
            "ximg": imgs32[core],
            "cblob": cblob,
        })
    return in_maps


def _jit_runner(nc):
    import jax
    from jax.sharding import Mesh, PartitionSpec, NamedSharding
    from jax.experimental.shard_map import shard_map
    from concourse import bass2jax
    bass2jax.install_neuronx_cc_hook()

    partition_name = nc.partition_id_tensor.name if nc.partition_id_tensor else None
    in_names, out_names, out_avals = [], [], []
    for alloc in nc.m.functions[0].allocations:
        if not isinstance(alloc, mybir.MemoryLocationSet):
            continue
        name = alloc.memorylocations[0].name
        if alloc.kind == "ExternalInput":
            if name != partition_name:
                in_names.append(name)
        elif alloc.kind == "ExternalOutput":
            out_names.append(name)
            out_avals.append(jax.core.ShapedArray(
                tuple(alloc.tensor_shape), mybir.dt.np(alloc.dtype)))
    all_in = in_names + out_names + ([partition_name] if partition_name else [])
    devices = jax.devices()[:N_CORES]
    mesh = Mesh(np.asarray(devices), ("core",))
    sh = NamedSharding(mesh, PartitionSpec("core"))

    def _body(*args):
        ops = list(args)
        if partition_name:
            ops.append(bass2jax.partition_id_tensor())
        outs = bass2jax._bass_exec_p.bind(
            *ops, out_avals=tuple(out_avals), in_names=tuple(all_in),
            out_names=tuple(out_names), lowering_input_output_aliases=(),
            sim_require_finite=True, sim_require_nnan=True, nc=nc)
        return tuple(outs)

    specs = (PartitionSpec("core"),) * (len(in_names) + len(out_names))
    ospec = (PartitionSpec("core"),) * len(out_names)
    f = jax.jit(shard_map(_body, mesh=mesh, in_specs=specs, out_specs=ospec,
                          check_rep=False))
    return f, in_names, out_avals, sh


def time_on_hw(inputs, n_lo=16, n_hi=256, iters=10):
    """Per-kernel HW time via an on-device For_i loop: slope of min wall
    between trip counts (includes ~2us loop back-edge per iteration)."""
    import time as _time
    import jax
    in_maps = _host_in_maps(inputs)
    mins = {}
    for loop_n in (n_lo, n_hi):
        nc = _get_program(inputs, loop_n=loop_n)
        f, in_names, out_avals, sh = _jit_runner(nc)
        cat = lambda n: np.concatenate(
            [np.asarray(in_maps[c][n]) for c in range(N_CORES)], axis=0)
        ci = [jax.device_put(cat(n), sh) for n in in_names]
        cz = [jax.device_put(
            np.zeros((N_CORES * a.shape[0], *a.shape[1:]), a.dtype), sh)
            for a in out_avals]
        jax.block_until_ready(f(*ci, *cz))
        best = float("inf")
        for _ in range(iters):
            t0 = _time.perf_counter()
            jax.block_until_ready(f(*ci, *cz))
            best = min(best, _time.perf_counter() - t0)
        mins[loop_n] = best
        print(f"[hw timing] loop_n={loop_n}: min wall {best*1e3:.2f} ms")
    slope_ns = (mins[n_hi] - mins[n_lo]) / (n_hi - n_lo) * 1e9
    print(f"[hw timing] -> {slope_ns:.0f} ns/kernel (incl ~2us loop overhead)")
    return slope_ns


def kernel(**inputs):
    global LAST_RESULTS
    j = np.asarray(inputs["joints"])
    assert j.shape == (B, 7), f"kernel hardcodes joints shape {(B, 7)}, got {j.shape}"
    nc = _get_program(inputs)
    in_maps = _host_in_maps(inputs)
    res = bass_utils.run_bass_kernel_spmd(nc, in_maps, core_ids=list(range(N_CORES)))
    LAST_RESULTS = res

    out = np.empty((B, 3), np.float32)
    for core in range(N_CORES):
        p = res.results[core]["out"]                   # [128, 768]
        # pack cols: [px(256) | py(256) | pz(256)], b_local = 256*p + m
        oc = p.reshape(128, 3, 256).transpose(0, 2, 1).reshape(BC, 3)
        out[core * BC:(core + 1) * BC] = oc
    return out


# revision 31
# speedup vs baseline: 5.1910x; 2.3914x over previous
"""Trainium2 Bass kernel for batched FK chain with tanh-MLP joint correction.

Math: per batch row,
    corr = tanh MLP_{7-15-15-7}(joints);  th = joints + off + corr
    M_j = DH(alpha_j, a_j, d_j, th_j);    out = (M_0 @ ... @ M_6)[:3, 3]
Factorization: M_j = A_j @ Rz(th_j) with A_j constant, and col 3 of M_6 is
constant, so the chain is 6 steps of (z-rotation + constant affine) on a
3-vector.

Distribution: pure data parallel, batch/8 = 32768 rows per NeuronCore.

Per-core pipeline (two batch halves, pipelined against each other):
  - host packs feature-major images [128, 2048]: partition q = 64h+8k+g
    (16 batch groups x 7 features, 8 gap rows), free n = 128c+p,
    batch row b = 256p + 16c + 8h + g; fp16 copy (joints+offset, offset
    folded via b1' = b1 - W1 @ off) for the MLP, fp32 copy (host
    range-reduced to [-pi, pi]) for the angle path
  - 3 MLP layers as fp16 block-pattern matmuls on PE (8 rows/cycle),
    tanh on ACT with per-partition bias; hidden activations fp16;
    L3's tanh split in two so transpose pairs overlap the second piece
  - PE transpose-mode fp32 matmul pairs accumulate xr.T + corr.T into
    PSUM (the theta add is free; pairs must be adjacent per region --
    start=False onto a closed PSUM group hangs the exec unit)
  - sin/cos via ACT Sin on half-angles: s2 = sin(th/2), s4 = sin(th/4);
    cos th = 1-2*s2^2, sin th = s2*(2-4*s4^2) (5 elementwise ops);
    tanh+sin+identity share one table set (silu_and_others, forced via a
    doctored table map so only one ACT table load happens)
  - trig planes live in merged [128, 1536] tiles (plane k at cols
    [256k, 256k+256)); each half's chain runs on its [128, 128]
    sub-planes with a per-half engine map (DVE carries the stt ops --
    scalar_tensor_tensor is DVE-only in the ISA; Pool takes t1/q1/q2,
    ACT Identity ops take h1's step-5 affines)
  - half 0's chain+combine overlap half 1's MLP on DVE/Pool while ACT
    streams half 1's tanh/sins; per-half output DMA, final x/y/z row
    DMAs interleaved with the last chain step
  - engine assignment and emission order tuned against TimelineSim;
    engines execute their queues in emission order, so op placement IS
    the schedule

Measured: rel err ~8.8e-4 vs fp32 reference; TimelineSim 36.7 us
single-shot (baseline 37.9); ~34 us/iter via the on-device For_i loop
(wall-clock noise is large).
"""

import numpy as np

import concourse.bass as bass
import concourse.tile as tile
from concourse import bacc, mybir
from concourse import bass_utils

N_CORES = 8
B = 262144
BC = B // N_CORES            # 32768 rows per core
C1 = 8                       # 128-col chunks in half 0 (psum: keep 8)
C2 = 16 - C1                 # chunks in half 1

F16 = mybir.dt.float16
F32 = mybir.dt.float32
AF = mybir.ActivationFunctionType
OP = mybir.AluOpType

C_BIAS1, C_BIAS2, C_BIAS3 = 0, 1, 2
# step-5 compound scalars (m, a) pairs, then generic steps j=0..4
C_ZERO = 3
C_M4, C_P2, C_M2, C_P1 = 10, 11, 12, 13
C_S5U1M, C_S5U1A = 4, 5
C_S5U3M, C_S5U3A = 6, 7
C_S5U5M, C_S5U5A = 8, 9
def _CJ(j, k):
    # k: 0=a, 1=ca, 2=sa, 3=dsa, 4=cad
    return 16 + 5 * j + k
NCONST = 48

# Engine codes: v=DVE, p=GPSIMD(Pool), a=ACT (ts-type ops only).
# Chain ops run once on merged [128, 256] planes; q/u ops may ride ACT
# (Identity with consts-blob scale/bias). scalar_tensor_tensor is DVE-only.
CFG = {
    # chain engine maps: merged mode uses ["ch"][None]; split mode uses
    # per-half maps ["ch"][0]/["ch"][1] (h0 chain hides under h1's MLP).
    "split_chain": True,
    "c1": 8,                  # chunks in half 0 (16 - c1 in half 1)
    "l1_split0": True,        # split h0's L1 tanh into [*,512] pieces
    "block_sins": False,      # sins per 4-chunk block (pipelines vs PE)
    "chain0_early": False,    # emit h0's chain before h1's MLP
    "ch": {
        None: {"t1": "p", "t2": "p", "t3": "v", "t4": "v", "yr": "v",
               "q1": "a", "q2": "a", "u1": "a", "u3": "a", "u5": "a",
               "xn": "v", "yn": "v", "zn": "v",
               "x5": "v", "y5": "v", "z5": "v"},
        0: {"t1": "p", "t2": "p", "t3": "v", "t4": "v", "yr": "v",
            "q1": "p", "q2": "p", "u1": "p", "u3": "p", "u5": "p",
            "xn": "v", "yn": "v", "zn": "v",
            "x5": "v", "y5": "v", "z5": "v"},
        1: {"t1": "p", "t2": "v", "t3": "v", "t4": "v", "yr": "v",
            "q1": "p", "q2": "p", "u1": "a", "u3": "a", "u5": "a",
            "xn": "v", "yn": "v", "zn": "v",
            "x5": "v", "y5": "v", "z5": "v"},
    },
    # per-half combine engine map + plane-group splits (descending k so
    # the chain's first-needed planes are ready soonest)
    "cb": {0: {"SQ": "v", "CT": "v", "SQ4": "v", "C2": "v", "ST": "v"},
           1: {"SQ": "v", "CT": "v", "SQ4": "v", "C2": "v", "ST": "v"}},
    "cb_groups": {0: [(0, 6)], 1: [(0, 6)]},
}


def _build_host_data(inputs):
    joints = np.asarray(inputs["joints"], np.float32)
    fk = np.asarray(inputs["fk_params"], np.float32)
    W1 = np.asarray(inputs["W1"], np.float32)
    b1 = np.asarray(inputs["b1"], np.float32)
    W2 = np.asarray(inputs["W2"], np.float32)
    b2 = np.asarray(inputs["b2"], np.float32)
    W3 = np.asarray(inputs["W3"], np.float32)
    b3 = np.asarray(inputs["b3"], np.float32)

    off = fk[:, 3]
    b1p = b1 - W1 @ off
    x_off = joints + off[None, :]          # [B, 7] fp32
    # exact host range-reduction for the angle path (Sin on ACT needs
    # [-pi, pi]; device uses half-angle identities so th = x_red + corr
    # stays in range)
    x_red = (np.remainder(x_off + np.pi, 2 * np.pi) - np.pi).astype(np.float32)

    # --- per-core feature-major images ---
    # batch row b = 256*p + 16*c + 8*h + g; partition q = 64*h + 8*k + g
    # img[q, 128*c + p] = src[b, k]
    def mkimg(src, dtype):
        out = []
        for core in range(N_CORES):
            jc = src[core * BC:(core + 1) * BC]            # [32768, 7]
            arr = jc.reshape(128, 16, 2, 8, 7)             # [p, c, h, g, k]
            arr = arr.transpose(2, 4, 3, 1, 0)             # [h, k, g, c, p]
            img = np.zeros((2, 8, 8, 16, 128), np.float32)
            img[:, :7] = arr
            out.append(np.ascontiguousarray(
                img.reshape(128, 2048)).astype(dtype))
        return out
    imgs16 = mkimg(x_off, np.float16)
    imgs32 = mkimg(x_red, np.float32)

    # --- block-pattern weights (fp16), packed into one [128, 304] blob ---
    blob = np.zeros((128, 304), np.float16)
    # L1: lhsT1[64h+8k+g, 15g+j] = W1[j, k]  (cols 0:120)
    for h in (0, 1):
        for k in range(7):
            for g in range(8):
                blob[64 * h + 8 * k + g, 15 * g:15 * g + 15] = W1[:, k]
    # L2: lhsT2[15g+i, 15g+j] = W2[j, i]  (cols 120:240)
    for g in range(8):
        blob[15 * g:15 * g + 15, 120 + 15 * g:120 + 15 * g + 15] = W2.T
    # L3: lhsT3[15g+i, 8k+g] = W3[k, i]  (cols 240:304)
    for g in range(8):
        for k in range(7):
            blob[15 * g:15 * g + 15, 240 + 8 * k + g] = W3[k, :]

    # --- constants blob [128, NCONST] fp32 (per-partition MLP biases) ---
    consts = np.zeros((128, NCONST), np.float32)
    for g in range(8):
        for j in range(15):
            consts[15 * g + j, C_BIAS1] = b1p[j]
            consts[15 * g + j, C_BIAS2] = b2[j]
    for h in (0, 1):
        for k in range(7):
            for g in range(8):
                consts[64 * h + 8 * k + g, C_BIAS3] = b3[k]

    alpha, a, d = fk[:, 0], fk[:, 1], fk[:, 2]
    ca, sa = np.cos(alpha), np.sin(alpha)
    t6 = np.array([a[6], -d[6] * sa[6], ca[6] * d[6]], np.float32)
    consts[:, C_M4] = -4.0
    consts[:, C_P2] = 2.0
    consts[:, C_M2] = -2.0
    consts[:, C_P1] = 1.0
    consts[:, C_S5U1M] = a[6]
    consts[:, C_S5U1A] = a[5]
    consts[:, C_S5U3M] = ca[5] * a[6]
    consts[:, C_S5U3A] = -sa[5] * t6[2] - d[5] * sa[5]
    consts[:, C_S5U5M] = sa[5] * a[6]
    consts[:, C_S5U5A] = ca[5] * t6[2] + ca[5] * d[5]
    for j in range(5):
        consts[:, _CJ(j, 0)] = a[j]
        consts[:, _CJ(j, 1)] = ca[j]
        consts[:, _CJ(j, 2)] = sa[j]
        consts[:, _CJ(j, 3)] = d[j] * sa[j]
        consts[:, _CJ(j, 4)] = ca[j] * d[j]

    id32 = np.ascontiguousarray(np.eye(128, dtype=np.float32))
    cblob = np.concatenate([
        blob.view(np.uint8).reshape(128, 608),
        consts.view(np.uint8).reshape(128, NCONST * 4),
        id32.view(np.uint8).reshape(128, 512),
    ], axis=1)
    return imgs16, imgs32, np.ascontiguousarray(cblob)


def _emit_program(nc, sc, reps=1, loop_n=0):
    dx16 = nc.dram_tensor("x16", [128, 2048], F16, kind="ExternalInput")
    dximg = nc.dram_tensor("ximg", [128, 2048], F32, kind="ExternalInput")
    CBYTES = 608 + NCONST * 4 + 512
    dcblob = nc.dram_tensor("cblob", [128, CBYTES], mybir.dt.uint8,
                            kind="ExternalInput")
    dout = nc.dram_tensor("out", [128, 768], F32, kind="ExternalOutput")
    from contextlib import ExitStack, nullcontext
    with tile.TileContext(nc) as tc, ExitStack() as ctx:
        cp = ctx.enter_context(tc.tile_pool(name="persist", bufs=1))
        ip = ctx.enter_context(tc.tile_pool(name="inp", bufs=2))
        hp = ctx.enter_context(tc.tile_pool(name="halfp", bufs=2))
        mlp_ps = ctx.enter_context(
            tc.tile_pool(name="mlpps", bufs=2, space="PSUM"))
        tp_ps = ctx.enter_context(
            tc.tile_pool(name="tpps", bufs=2, space="PSUM"))
        chp = ctx.enter_context(tc.tile_pool(name="chain", bufs=4))

        cblob = cp.tile([128, CBYTES], mybir.dt.uint8, tag="cblob")
        lhs = cblob[:, 0:608].bitcast(F16)
        consts = cblob[:, 608:608 + NCONST * 4].bitcast(F32)
        id32 = cblob[:, 608 + NCONST * 4:CBYTES].bitcast(F32)

        # hoist the ACT table load under the input DMAs
        warm = cp.tile([128, 1], F32, tag="warm")
        nc.vector.memset(warm[:], 0.0)
        nc.scalar.activation(warm[:], warm[:], AF.Tanh, bias=0.0)

        def cv(col, parts=128):
            return consts[0:parts, col:col + 1]

        def eng(code):
            return nc.vector if code == "v" else nc.gpsimd

        def sq_op(code, out, in_):
            """out = in_^2 on DVE/Pool tt or ACT Square."""
            if code == "a":
                nc.scalar.activation(out, in_, AF.Square, bias=0.0)
            else:
                eng(code).tensor_tensor(out, in_, in_, OP.mult)

        def ts_op(code, out, in_, mul, add, mcol=None, acol=None):
            """out = in_*mul + add on DVE/Pool ts or ACT identity.
            For the ACT path, mcol/acol are consts-blob columns holding the
            same scalars (ACT bias/scale must be APs)."""
            if code == "a":
                nc.scalar.activation(out, in_, AF.Identity,
                                     bias=cv(acol), scale=cv(mcol))
            else:
                eng(code).tensor_scalar(out, in_, float(mul), float(add),
                                        OP.mult, OP.add)

        # Merged trig tiles span both halves: plane k at cols
        # [256k, 256k+256), col = 256k + 16cg + 8h + g, cg = global chunk.
        def halfview(t, c0, C):
            return t.rearrange("p (k cg h g) -> p k cg h g",
                               k=6, cg=16, h=2, g=8)[:, :, c0:c0 + C]

        def mlp_half(x16h, xrh, hf, C, l1_split=False, block_sins=None):
            """MLP + transposes for one half; returns the theta psum."""
            NF = 128 * C
            half = 512 * ((NF // 2) // 512) if NF // 2 >= 512 else NF
            ps2 = tp_ps.tile([128, NF], F32, tag="tpps", name="ps2")
            h1 = hp.tile([128, 2 * NF], F16, tag="h1", name="h1")
            h2 = hp.tile([128, 2 * NF], F16, tag="h2", name="h2")
            corr = hp.tile([128, NF], F32, tag="corr", name="corr")

            def mm_slices(n0, n1):
                out = []
                o = n0
                while o < n1:
                    w = min(512, n1 - o)
                    out.append((o, w))
                    o += w
                return out

            # L1: K=64 per h-half (gap layout), M=120
            for h in (0, 1):
                ps = mlp_ps.tile([128, NF], F32, tag="mlpps", name="ps")
                pieces = [(0, half), (half, NF)] if l1_split else [(0, NF)]
                for (p0, p1) in pieces:
                    if p0 == p1:
                        continue
                    for (o, w) in mm_slices(p0, p1):
                        nc.tensor.matmul(
                            ps[0:120, o:o + w],
                            lhs[64 * h:64 * h + 64, 0:120],
                            x16h[64 * h:64 * h + 64, o:o + w],
                            start=True, stop=True, tile_position=(64 * h, 0))
                    nc.scalar.activation(
                        h1[0:120, NF * h + p0:NF * h + p1],
                        ps[0:120, p0:p1], AF.Tanh, bias=cv(C_BIAS1, 120))
            # L2
            for h in (0, 1):
                ps = mlp_ps.tile([128, NF], F32, tag="mlpps", name="ps")
                for (o, w) in mm_slices(0, NF):
                    nc.tensor.matmul(
                        ps[0:120, o:o + w],
                        lhs[0:120, 120:240],
                        h1[0:120, NF * h + o:NF * h + o + w],
                        start=True, stop=True)
                nc.scalar.activation(
                    h2[0:120, NF * h:NF * h + NF],
                    ps[0:120, :], AF.Tanh, bias=cv(C_BIAS2, 120))
            # L3: both h-halves stacked on psum partitions via col groups;
            # tanh split in two; second L3 matmul group is emitted before the
            # first transpose batch so ACT L3a/L3b run back-to-back
            ps = mlp_ps.tile([128, NF], F32, tag="mlpps", name="ps")

            def l3mm(n0, n1):
                for h in (0, 1):
                    for (o, w) in mm_slices(n0, n1):
                        nc.tensor.matmul(
                            ps[64 * h:64 * h + 64, o:o + w],
                            lhs[0:120, 240:304],
                            h2[0:120, NF * h + o:NF * h + o + w],
                            start=True, stop=True, tile_position=(0, 64 * h))

            def tp(ca, cb):
                # transpose pairs (xr + corr accumulate) -> th = xr + corr
                for c in range(ca, cb):
                    nc.tensor.matmul(
                        ps2[:, 128 * c:128 * c + 128],
                        xrh[:, 128 * c:128 * c + 128], id32[:],
                        is_transpose=True, start=True, stop=False)
                    nc.tensor.matmul(
                        ps2[:, 128 * c:128 * c + 128],
                        corr[:, 128 * c:128 * c + 128], id32[:],
                        is_transpose=True, start=False, stop=True)
            l3mm(0, half)
            nc.scalar.activation(corr[:, 0:half], ps[:, 0:half],
                                 AF.Tanh, bias=cv(C_BIAS3))
            l3mm(half, NF)
            tp(0, half // 128)
            nc.scalar.activation(corr[:, half:NF], ps[:, half:NF],
                                 AF.Tanh, bias=cv(C_BIAS3))
            if block_sins:
                block_sins(ps2, 0, half // 128)
            tp(half // 128, C)
            if block_sins:
                block_sins(ps2, half // 128, C)
            return ps2

        def sin_part(hf, ps2, c0, C, S2, S4, CT, ST, SQ, CB2, b0=0, b1=None):
            """Sins + combine for chunk sub-range [c0+b0, c0+b1)."""
            em = CFG["cb"][hf]
            if b1 is None:
                b1 = C
            # sins: s2 = sin(th/2), s4 = sin(th/4); into merged plane layout
            in_all = ps2[:, :].rearrange(
                "p (c h k g) -> p k c h g", c=C, h=2, k=8, g=8)[:, 0:6,
                                                                b0:b1]
            nc.scalar.activation(halfview(S2, c0 + b0, b1 - b0), in_all,
                                 AF.Sin, bias=0.0, scale=0.5)
            nc.scalar.activation(halfview(S4, c0 + b0, b1 - b0), in_all,
                                 AF.Sin, bias=0.0, scale=0.25)
            # combine: CT = 1-2*s2^2 ; ST = s2*(2-4*s4^2).
            # The s4 branch (SQ4/C2) may ride ACT (Square + Identity) --
            # after the last sins ACT is otherwise idle, freeing DVE.
            groups = CFG["cb_groups"][hf]
            for (k0, k1) in groups:
                def hv(t):
                    return halfview(t, c0 + b0, b1 - b0)[:, k0:k1]
                s2v, s4v, sqv, c2v = hv(S2), hv(S4), hv(SQ), hv(CB2)
                ctv, stv = hv(CT), hv(ST)
                # SQ (sin2-dep) first; the s4 branch may ride ACT
                if em["SQ4"] == "a":
                    sq_op("a", c2v, s4v)
                    ts_op(em["C2"], c2v, c2v, -4.0, 2.0, C_M4, C_P2)
                    sq_op(em["SQ"], sqv, s2v)
                    ts_op(em["CT"], ctv, sqv, -2.0, 1.0, C_M2, C_P1)
                else:
                    sq_op(em["SQ"], sqv, s2v)
                    ts_op(em["CT"], ctv, sqv, -2.0, 1.0, C_M2, C_P1)
                    sq_op(em["SQ4"], c2v, s4v)
                    ts_op(em["C2"], c2v, c2v, -4.0, 2.0, C_M4, C_P2)
                eng(em["ST"]).tensor_tensor(stv, s2v, c2v, OP.mult)

        def chain_run(CT, ST, pack, hf=None, c0=0, C=16, row_dma=None):
            """One FK chain pass over chunk range [c0, c0+C).
            row_dma(row, sl): called right after the final x/y/z write so the
            output DMAs overlap the last step (row: 0=x,1=y,2=z)."""
            em = CFG["ch"][hf]
            W = 16 * C
            off = 16 * c0

            def ctj(j):
                return CT[:, 256 * j + off:256 * j + off + W]

            def stj(j):
                return ST[:, 256 * j + off:256 * j + off + W]

            def ch(tag):
                return chp.tile([128, W], F16, tag=tag + str(hf), name=tag)

            # step 5 (innermost): affine in (c5, s5)
            u1 = ch("u1")
            ts_op(em["u1"], u1, ctj(5), sc["s5u1m"], sc["s5u1a"],
                  C_S5U1M, C_S5U1A)
            u3 = ch("u3")
            ts_op(em["u3"], u3, stj(5), sc["s5u3m"], sc["s5u3a"],
                  C_S5U3M, C_S5U3A)
            u5 = ch("u5")
            ts_op(em["u5"], u5, stj(5), sc["s5u5m"], sc["s5u5a"],
                  C_S5U5M, C_S5U5A)
            x = ch("x")
            eng(em["x5"]).scalar_tensor_tensor(
                x, stj(5), sc["s5xm"], u1, OP.mult, OP.add)
            y = ch("y")
            eng(em["y5"]).scalar_tensor_tensor(
                y, ctj(5), sc["s5ym"], u3, OP.mult, OP.add)
            z = ch("z")
            eng(em["z5"]).scalar_tensor_tensor(
                z, ctj(5), sc["s5zm"], u5, OP.mult, OP.add)

            for j in (4, 3, 2, 1, 0):
                last = j == 0
                a_j, ca_j, sa_j = sc[f"a{j}"], sc[f"ca{j}"], sc[f"sa{j}"]
                dsa_j, cad_j = sc[f"dsa{j}"], sc[f"cad{j}"]
                # q's early (dep: z only); t1/t2 on Pool feed xn late
                q1 = ch("q1")
                ts_op(em["q1"], q1, z, sa_j, dsa_j, _CJ(j, 2), _CJ(j, 3))
                q2 = ch("q2")
                ts_op(em["q2"], q2, z, ca_j, cad_j, _CJ(j, 1), _CJ(j, 4))
                t1 = ch("t1")
                eng(em["t1"]).tensor_tensor(t1, x, ctj(j), OP.mult)
                t2 = ch("t2")
                eng(em["t2"]).tensor_tensor(t2, y, stj(j), OP.mult)
                t3 = ch("t3")
                eng(em["t3"]).tensor_tensor(t3, x, stj(j), OP.mult)
                t4 = ch("t4")
                eng(em["t4"]).tensor_tensor(t4, y, ctj(j), OP.mult)
                yr = ch("yr")
                eng(em["yr"]).tensor_tensor(yr, t3, t4, OP.add)
                yn = pack[:, 256 + off:256 + off + W] if last else ch("y")
                eng(em["yn"]).scalar_tensor_tensor(
                    yn, yr, ca_j, q1, OP.mult, OP.subtract)
                if last and row_dma:
                    row_dma(1)
                zn = pack[:, 512 + off:512 + off + W] if last else ch("z")
                eng(em["zn"]).scalar_tensor_tensor(
                    zn, yr, sa_j, q2, OP.mult, OP.add)
                if last and row_dma:
                    row_dma(2)
                xn = pack[:, off:off + W] if last else ch("x")
                eng(em["xn"]).scalar_tensor_tensor(
                    xn, t1, a_j, t2, OP.add, OP.subtract)
                if last and row_dma:
                    row_dma(0)
                x, y, z = xn, yn, zn

        # PE warm-up: dummy matmuls on a memset tile so the PE clock ramps
        # while the input DMAs are in flight.
        wm16 = cp.tile([128, 512], F16, tag="wm16")
        nc.vector.memset(wm16[:], 0.0)
        wmps = mlp_ps.tile([128, 1024], F32, tag="mlpps", name="wmps")
        for _w in range(3):
            nc.tensor.matmul(wmps[:, 0:512], wm16[0:64, 0:128],
                             wm16[0:64, :], start=True, stop=True)

        loop_ctx = tc.For_i(0, loop_n, 1) if loop_n else nullcontext()
        first = True
        with loop_ctx:
          for _rep in range(reps):
              NF1, NF2 = 128 * C1, 128 * C2
              x16a = ip.tile([128, NF1], F16, tag="x16a", name="x16a")
              x16b = ip.tile([128, NF2], F16, tag="x16b", name="x16b")
              xra = ip.tile([128, NF1], F32, tag="xra", name="xra")
              xrb = ip.tile([128, NF2], F32, tag="xrb", name="xrb")
              pack = ip.tile([128, 768], F32, tag="pack", name="pack")
              S2 = ip.tile([128, 1536], F16, tag="S2", name="S2")
              S4 = ip.tile([128, 1536], F16, tag="S4", name="S4")
              CT = ip.tile([128, 1536], F16, tag="CT", name="CT")
              ST = ip.tile([128, 1536], F16, tag="ST", name="ST")
              SQ = ip.tile([128, 1536], F16, tag="SQ", name="SQ")
              CB2 = ip.tile([128, 1536], F16, tag="CB2", name="CB2")
              # first DMA split so h0's first L1 piece starts sooner
              nc.sync.dma_start(x16a[:, 0:512], dx16.ap()[:, 0:512])
              if first:
                  nc.sync.dma_start(cblob[:], dcblob.ap())
                  first = False
              nc.sync.dma_start(x16a[:, 512:NF1], dx16.ap()[:, 512:NF1])
              nc.sync.dma_start(x16b[:], dx16.ap()[:, NF1:2048])
              nc.sync.dma_start(xra[:], dximg.ap()[:, 0:NF1])
              nc.sync.dma_start(xrb[:], dximg.ap()[:, NF1:2048])
              pv = pack[:, :].rearrange("p (c h) -> p c h", c=3, h=256)
              dv = dout.ap().rearrange("p (c h) -> p c h", c=3, h=256)
              def mk_bs(hf, c0, C):
                  def bs(ps2, b0, b1):
                      sin_part(hf, ps2, c0, C, S2, S4, CT, ST, SQ, CB2,
                               b0, b1)
                  return bs
              BS = CFG["block_sins"]
              ps2a = mlp_half(x16a, xra, 0, C1, l1_split=CFG["l1_split0"],
                              block_sins=mk_bs(0, 0, C1) if BS else None)
              if not BS:
                  sin_part(0, ps2a, 0, C1, S2, S4, CT, ST, SQ, CB2)
              if CFG["split_chain"]:
                  W1 = 16 * C1
                  def h1_mlp_sins():
                      ps2b = mlp_half(x16b, xrb, 1, C2,
                                      block_sins=mk_bs(1, C1, C2) if BS
                                      else None)
                      if not BS:
                          sin_part(1, ps2b, C1, C2, S2, S4, CT, ST, SQ, CB2)
                  def row_dma(row):
                      nc.sync.dma_start(dv[:, row:row + 1, W1:256],
                                        pv[:, row:row + 1, W1:256])
                  if CFG["chain0_early"]:
                      chain_run(CT, ST, pack, 0, 0, C1)
                      nc.sync.dma_start(dv[:, :, 0:W1], pv[:, :, 0:W1])
                      h1_mlp_sins()
                  else:
                      h1_mlp_sins()
                      chain_run(CT, ST, pack, 0, 0, C1)
                      nc.sync.dma_start(dv[:, :, 0:W1], pv[:, :, 0:W1])
                  chain_run(CT, ST, pack, 1, C1, C2, row_dma=row_dma)
              else:
                  ps2b = mlp_half(x16b, xrb, 1, C2,
                                  block_sins=mk_bs(1, C1, C2) if BS else None)
                  if not BS:
                      sin_part(1, ps2b, C1, C2, S2, S4, CT, ST, SQ, CB2)
                  chain_run(CT, ST, pack)
                  nc.sync.dma_start(dv[:, :, 0:128], pv[:, :, 0:128])
                  nc.sync.dma_start(dv[:, :, 128:256], pv[:, :, 128:256])


_PROG_CACHE = {}


def _baked_scalars(inputs):
    fk = np.asarray(inputs["fk_params"], np.float32)
    alpha, a, d = fk[:, 0], fk[:, 1], fk[:, 2]
    ca, sa = np.cos(alpha), np.sin(alpha)
    t6 = np.array([a[6], -d[6] * sa[6], ca[6] * d[6]], np.float32)
    sc = {
        "s5u1m": a[6], "s5u1a": a[5], "s5xm": -t6[1],
        "s5u3m": ca[5] * a[6], "s5u3a": -sa[5] * t6[2] - d[5] * sa[5],
        "s5ym": ca[5] * t6[1],
        "s5u5m": sa[5] * a[6], "s5u5a": ca[5] * t6[2] + ca[5] * d[5],
        "s5zm": sa[5] * t6[1],
    }
    for j in range(5):
        sc[f"a{j}"] = a[j]
        sc[f"ca{j}"] = ca[j]
        sc[f"sa{j}"] = sa[j]
        sc[f"dsa{j}"] = d[j] * sa[j]
        sc[f"cad{j}"] = ca[j] * d[j]
    return {k: float(np.float32(v)) for k, v in sc.items()}


def _cfg_key():
    def frz(v):
        if isinstance(v, dict):
            return tuple(sorted(((str(k), frz(x)) for k, x in v.items())))
        if isinstance(v, list):
            return tuple(v)
        return v
    return frz(CFG)


def _get_program(inputs, reps=1, loop_n=0):
    sc = _baked_scalars(inputs)
    key = (tuple(sorted(sc.items())), reps, loop_n, _cfg_key())
    if key in _PROG_CACHE:
        return _PROG_CACHE[key]
    nc = bacc.Bacc("TRN2", target_bir_lowering=False, debug=False,
                   enable_asserts=False)
    _emit_program(nc, sc, reps=reps, loop_n=loop_n)

    # Force Tanh/Sin (and Identity) to resolve to the one table set
    # containing all of them (silu_and_others), so the kernel pays a single
    # ACT table load.
    import concourse.bacc as bacc_mod
    from concourse.hw_specs import get_activation_tables
    orig_fn = bacc_mod.get_activation_tables
    tabs = get_activation_tables(nc.m.arch)
    trig = {AF.Tanh, AF.Sin, AF.Identity}
    doctored = {
        name: (set(funcs) if name == "silu_and_others" else set(funcs) - trig)
        for name, funcs in tabs.items()
    }
    bacc_mod.get_activation_tables = lambda arch: doctored
    try:
        nc.compile()
    finally:
        bacc_mod.get_activation_tables = orig_fn

    _PROG_CACHE[key] = nc
    return nc


LAST_RESULTS = None  # BassKernelResults of the most recent run (for test.py)


def _host_in_maps(inputs):
    imgs16, imgs32, cblob = _build_host_data(inputs)
    in_maps = []
    for core in range(N_CORES):
        in_maps.append({
            "x16": imgs16[core],
            "ximg": imgs32[core],
            "cblob": cblob,
        })
    return in_maps


def _jit_runner(nc):
    import jax
    from jax.sharding import Mesh, PartitionSpec, NamedSharding
    from jax.experimental.shard_map import shard_map
    from concourse import bass2jax
    bass2jax.install_neuronx_cc_hook()

    partition_name = nc.partition_id_tensor.name if nc.partition_id_tensor else None
    in_names, out_names, out_avals = [], [], []
    for alloc in nc.m.functions[0].allocations:
        if not isinstance(alloc, mybir.MemoryLocationSet):
            continue
        name = alloc.memorylocations[0].name
        if alloc.kind == "ExternalInput":
            if name != partition_name:
                in_names.append(name)
        elif alloc.kind == "ExternalOutput":
            out_names.append(name)
            out_avals.append(jax.core.ShapedArray(
                tuple(alloc.tensor_shape), mybir.dt.np(alloc.dtype)))
    all_in = in_names + out_names + ([partition_name] if partition_name else [])
    devices = jax.devices()[:N_CORES]
    mesh = Mesh(np.asarray(devices), ("core",))
    sh = NamedSharding(mesh, PartitionSpec("core"))

    def _body(*args):
        ops = list(args)
        if partition_name:
            ops.append(bass2jax.partition_id_tensor())
        outs = bass2jax._bass_exec_p.bind(
            *ops, out_avals=tuple(out_avals), in_names=tuple(all_in),
            out_names=tuple(out_names), lowering_input_output_aliases=(),
            sim_require_finite=True, sim_require_nnan=True, nc=nc)
        return tuple(outs)

    specs = (PartitionSpec("core"),) * (len(in_names) + len(out_names))
    ospec = (PartitionSpec("core"),) * len(out_names)
    f = jax.jit(shard_map(_body, mesh=mesh, in_specs=specs, out_specs=ospec,
                          check_rep=False))
    return f, in_names, out_avals, sh


def time_on_hw(inputs, n_lo=16, n_hi=256, iters=10):
    """Per-kernel HW time via an on-device For_i loop: slope of min wall
    between trip counts (includes ~2us loop back-edge per iteration)."""
    import time as _time
    import jax
    in_maps = _host_in_maps(inputs)
    mins = {}
    for loop_n in (n_lo, n_hi):
        nc = _get_program(inputs, loop_n=loop_n)
        f, in_names, out_avals, sh = _jit_runner(nc)
        cat = lambda n: np.concatenate(
            [np.asarray(in_maps[c][n]) for c in range(N_CORES)], axis=0)
        ci = [jax.device_put(cat(n), sh) for n in in_names]
        cz = [jax.device_put(
            np.zeros((N_CORES * a.shape[0], *a.shape[1:]), a.dtype), sh)
            for a in out_avals]
        jax.block_until_ready(f(*ci, *cz))
        best = float("inf")
        for _ in range(iters):
            t0 = _time.perf_counter()
            jax.block_until_ready(f(*ci, *cz))
            best = min(best, _time.perf_counter() - t0)
        mins[loop_n] = best
        print(f"[hw timing] loop_n={loop_n}: min wall {best*1e3:.2f} ms")
    slope_ns = (mins[n_hi] - mins[n_lo]) / (n_hi - n_lo) * 1e9
    print(f"[hw timing] -> {slope_ns:.0f} ns/kernel (incl ~2us loop overhead)")
    return slope_ns


def kernel(**inputs):
    global LAST_RESULTS
    j = np.asarray(inputs["joints"])
    assert j.shape == (B, 7), f"kernel hardcodes joints shape {(B, 7)}, got {j.shape}"
    nc = _get_program(inputs)
    in_maps = _host_in_maps(inputs)
    res = bass_utils.run_bass_kernel_spmd(nc, in_maps, core_ids=list(range(N_CORES)))
    LAST_RESULTS = res

    out = np.empty((B, 3), np.float32)
    for core in range(N_CORES):
        p = res.results[core]["out"]                   # [128, 768]
        # pack cols: [px(256) | py(256) | pz(256)], b_local = 256*p + m
        oc = p.reshape(128, 3, 256).transpose(0, 2, 1).reshape(BC, 3)
        out[core * BC:(core + 1) * BC] = oc
    return out
